# revision 23
# baseline (speedup 1.0000x reference)
"""JumpAttention Trainium2 kernel (bf16 main path, fp8 agg garnish).

Problem: B=16, S=1024, H=2048, D=256.
  Q/K/K2/V = hs @ W*, 3 biased attentions + 2 aggregation attentions,
  triadic-weighted combine, output projection by Wo.

Strategy:
  - Data-parallel over batch: 2 batches per core on 8 cores; weights and
    pos_bias replicated.
  - bf16 matmuls with fp32 PSUM accumulation on the main path (fp8 there
    fails the accuracy gate: diffuse attention averages the signal down
    as fast as the noise, so per-element fp8 noise stays 2-4% of the
    output).  The aggregation branches are nearly flat softmaxes and
    tolerate fp8: their P tiles and K/K2 value tables run fp8-e4m3 with
    DoubleRow perf mode (half the PE instructions and cycles there).
  - Scores are computed transposed (s^T[k, q]) so that exp(s^T) directly
    serves as the stationary operand of the P@V matmul - no P transpose.
  - Softmax has no max-subtraction; exp(s - 3) keeps the fp8 agg P in
    range (the shift cancels in softmax); the denominator comes from a
    ones-column appended to the value table (row-sums appear as one
    extra output column of the P@V matmul).
  - pos_bias folded multiplicatively: exp(s + b) = exp(s) * exp(b), with
    exp(pos_bias^T) precomputed on host in bf16 (the multiply runs on
    DVE in its 2x all-SBUF bf16 mode).
  - softmax(triadic_weight) computed on host, baked in as immediates.
  - Two-batch software pipeline: emission is interleaved in two lanes so
    batch 1's projection work (PE/DVE, no Act) fills the engine gaps
    under batch 0's attention (Act-heavy), and each batch's final
    projection is fused into its attention units (no drain tail).
  - Constants (weights, exp-bias table, identity) load outside the
    timing loop; build_program(reps=N) wraps the per-iteration body in
    an on-device For_i loop so test.py can measure per-iteration HW time
    with host dispatch overhead amortized away.
"""

import os
from contextlib import ExitStack

import numpy as np
import ml_dtypes

B, S, H, D = 16, 1024, 2048, 256
NCORES = 8
BPC = B // NCORES  # batches per core
P = 128
HT = H // P   # 16 h-tiles
KT = S // P   # 8 s-tiles
DT = D // P   # 2 d-tiles
NQ = 512      # moving free dim (q) chunk
QC = S // NQ  # 2 q chunks
HC = H // NQ  # 4 h chunks in final projection
XG = 8        # h-tiles per x load group
DP = D + 2    # value-table width: D cols + ones col + pad (8B-aligned PSUM rows)

EXPB = -3.0   # exp argument shift (cancels in softmax; keeps fp8 P in range)

LAST_RESULTS = None


def _consts(nc, tc, ctx, mybir, make_identity, handles, w_tri):
    """Pools + loop-invariant constants (emitted outside the reps loop)."""
    xT_h, xT8_h, wq_h, wk_h, wk2_h, wv_h, wo_h, ebT_h, out_h = handles
    dt = mybir.dt
    bf16 = dt.bfloat16
    f32 = dt.float32
    fp8 = dt.float8e4

    pools = dict(
        consts=ctx.enter_context(tc.tile_pool(name="consts", bufs=1)),
        xpool=ctx.enter_context(tc.tile_pool(name="xpool", bufs=2)),
        actp=ctx.enter_context(tc.tile_pool(name="actp", bufs=1)),
        ppool=ctx.enter_context(tc.tile_pool(name="ppool", bufs=1)),
        tmpp=ctx.enter_context(tc.tile_pool(name="tmpp", bufs=4)),
        psA=ctx.enter_context(tc.tile_pool(name="psA", bufs=3, space="PSUM")),
        psO=ctx.enter_context(tc.tile_pool(name="psO", bufs=4, space="PSUM")),
        psT=ctx.enter_context(tc.tile_pool(name="psT", bufs=1, space="PSUM")),
    )
    consts = pools["consts"]

    wq_sb = consts.tile([P, HT, D], bf16, name="wq_sb")
    wk_sb = consts.tile([P, HT, D], bf16, name="wk_sb")
    wk2_sb = consts.tile([P, HT, D], fp8, name="wk2_sb")
    wv_sb = consts.tile([P, HT, D], bf16, name="wv_sb")
    for t_, h_ in ((wq_sb, wq_h), (wk_sb, wk_h), (wk2_sb, wk2_h), (wv_sb, wv_h)):
        nc.sync.dma_start(out=t_, in_=h_[:].rearrange("(t p) d -> p t d", p=P))
    wo_sb = consts.tile([P, DT, H], bf16, name="wo_sb")
    nc.sync.dma_start(out=wo_sb, in_=wo_h[:].rearrange("(t p) h -> p t h", p=P))
    ebT_sb = consts.tile([P, KT, S], bf16, name="ebT_sb")
    nc.sync.dma_start(out=ebT_sb, in_=ebT_h[:].rearrange("(t p) q -> p t q", p=P))
    identb = consts.tile([P, P], bf16, name="identb")
    make_identity(nc, identb)
    ident8 = consts.tile([P, P], fp8, name="ident8")
    nc.vector.tensor_copy(ident8, identb)
    ebias = consts.tile([P, 1], f32, name="ebias")
    nc.gpsimd.memset(ebias, EXPB)
    wconst = consts.tile([P, 4], f32, name="wconst")
    for _i, _w in enumerate((float(w_tri[0]), float(w_tri[1]),
                             float(w_tri[2]), 0.0)):
        nc.gpsimd.memset(wconst[:, _i:_i + 1], _w)
    cst = dict(wq_sb=wq_sb, wk_sb=wk_sb, wk2_sb=wk2_sb, wv_sb=wv_sb,
               wo_sb=wo_sb, ebT_sb=ebT_sb, identb=identb, ident8=ident8,
               ebias=ebias, wconst=wconst)
    return pools, cst



def _x_loads(nc, xpool, mybir, xT_h, xT8_h, b):
    xgs, xgs8 = [], []
    for g in range(HT // XG):
        xg = xpool.tile([P, XG, S], mybir.dt.bfloat16, name="xg", tag="xg")
        nc.sync.dma_start(
            out=xg,
            in_=xT_h[b, g * XG * P:(g + 1) * XG * P, :].rearrange(
                "(t p) q -> p t q", p=P
            ),
        )
        xgs.append(xg)
        xg8 = xpool.tile([P, XG, S], mybir.dt.float8e4, name="xg8", tag="xg8")
        nc.sync.dma_start(
            out=xg8,
            in_=xT8_h[b, g * XG * P:(g + 1) * XG * P, :].rearrange(
                "(t p) q -> p t q", p=P
            ),
        )
        xgs8.append(xg8)
    return xgs, xgs8


def _body(nc, tc, mybir, handles, pools, cst):
    """One full iteration of the kernel (both batches)."""
    xT_h, xT8_h, wq_h, wk_h, wk2_h, wv_h, wo_h, ebT_h, out_h = handles
    dt = mybir.dt
    bf16 = dt.bfloat16
    f32 = dt.float32
    fp8 = dt.float8e4
    DR = mybir.MatmulPerfMode.DoubleRow
    Exp = mybir.ActivationFunctionType.Exp
    SSC = float(D) ** -0.5

    xpool, actp, ppool, tmpp = (pools[k] for k in
                                ("xpool", "actp", "ppool", "tmpp"))
    psA, psO, psT = (pools[k] for k in ("psA", "psO", "psT"))
    (wq_sb, wk_sb, wk2_sb, wv_sb, wo_sb, ebT_sb, identb, ident8, ebias,
     wconst) = (
        cst[k] for k in ("wq_sb", "wk_sb", "wk2_sb", "wv_sb", "wo_sb",
                         "ebT_sb", "identb", "ident8", "ebias", "wconst"))

    def gen_A(b, xgs, xgs8, T):
        """x projections, V, K'/K2' fp8 row tables. No Act-engine work."""
        xts = [xgs[h_t // XG][:, h_t % XG, :] for h_t in range(HT)]

        kT = actp.tile([P, DT, S], bf16, name="kT", tag="kT", bufs=2)
        qT = actp.tile([P, DT, S], bf16, name="qT", tag="qT", bufs=2)
        T.update(qT=qT, kT=kT)

        def proj_cols(dst, w_sb):
            for d_t in range(DT):
                for q_c in range(QC):
                    ps = psA.tile([P, NQ], f32, name="ps_proj", tag="psA")
                    for h_t in range(HT):
                        nc.tensor.matmul(
                            ps,
                            lhsT=w_sb[:, h_t, d_t * P:(d_t + 1) * P],
                            rhs=xts[h_t][:, q_c * NQ:(q_c + 1) * NQ],
                            start=(h_t == 0),
                            stop=(h_t == HT - 1),
                        )
                    nc.vector.tensor_copy(dst[:, d_t, q_c * NQ:(q_c + 1) * NQ], ps)

        kT8 = actp.tile([P, DT, S], fp8, name="kT8", tag="kT8", bufs=2)
        k2T8 = actp.tile([P, DT, S], fp8, name="k2T8", tag="k2T8", bufs=2)
        T.update(kT8=kT8, k2T8=k2T8)
        proj_cols(kT, wk_sb)
        nc.vector.tensor_copy(kT8, kT)
        yield
        # K2 projection fully in fp8 DoubleRow (x8 @ Wk2_8): K2 only feeds
        # fp8 consumers (agg scores + fp8 value table), which tolerate the
        # extra quantization noise; halves this projection's PE cycles.
        for d_t in range(DT):
            for q_c in range(QC):
                ps = psA.tile([P, NQ], f32, name="ps_proj8", tag="psA")
                for hp in range(HT // 2):
                    g, j = hp // (XG // 2), hp % (XG // 2)
                    nc.tensor.matmul(
                        ps,
                        lhsT=wk2_sb[:, 2 * hp:2 * hp + 2, d_t * P:(d_t + 1) * P],
                        rhs=xgs8[g][:, 2 * j:2 * j + 2, q_c * NQ:(q_c + 1) * NQ],
                        start=(hp == 0),
                        stop=(hp == HT // 2 - 1),
                        perf_mode=DR,
                    )
                nc.vector.tensor_copy(
                    k2T8[:, d_t, q_c * NQ:(q_c + 1) * NQ], ps
                )
        yield

        # K', K2' row-layout fp8 value tables (+ ones col) for the agg PV;
        # transposes of K^T/K2^T, fp8 conversion in the copy.
        kP = actp.tile([P, KT, DP], fp8, name="kP", tag="kP", bufs=2)
        k2P = actp.tile([P, KT, DP], fp8, name="k2P", tag="k2P", bufs=2)
        T.update(kP=kP, k2P=k2P)
        for s_t in range(KT):
            pt = psT.tile([P, DT, P], bf16, name="pt_k", tag="psT")
            for d_t in range(DT):
                nc.tensor.transpose(
                    pt[:, d_t, :], kT[:, d_t, s_t * P:(s_t + 1) * P], identb
                )
            nc.vector.tensor_copy(kP[:, s_t, :D], pt)
        nc.gpsimd.memset(kP[:, :, D:DP], 1.0)
        for s_t in range(KT):
            # fp8 transpose-mode requires output element step 2
            pt = psT.tile([P, DT, 2 * P], fp8, name="pt_k8", tag="psT")
            for d_t in range(DT):
                nc.tensor.transpose(
                    pt[:, d_t, 0:2 * P:2],
                    k2T8[:, d_t, s_t * P:(s_t + 1) * P], ident8
                )
            nc.vector.tensor_copy(k2P[:, s_t, :D], pt[:, :, 0:2 * P:2])
        nc.gpsimd.memset(k2P[:, :, D:DP], 1.0)
        yield

        proj_cols(qT, wq_sb)
        # fp8 Q for the jump-branch scores (their logits are tiny -- K_agg
        # rows are diffuse softmax averages -- so fp8 noise is negligible
        # there; the std branch keeps bf16 Q).
        q8 = actp.tile([P, DT, S], fp8, name="q8", tag="q8", bufs=2)
        nc.vector.tensor_copy(q8, qT)
        T.update(q8=q8)
        yield

        # V' : [s-part, d + ones-column] bf16 (x as stationary)
        vP = actp.tile([P, KT, DP], bf16, name="vP", tag="vP", bufs=2)
        T.update(vP=vP)
        for s_t in range(KT):
            ps = psA.tile([P, NQ], f32, name="ps_v", tag="psA")
            for h_t in range(HT):
                nc.tensor.matmul(
                    ps[:, :D],
                    lhsT=xts[h_t][:, s_t * P:(s_t + 1) * P],
                    rhs=wv_sb[:, h_t, :],
                    start=(h_t == 0),
                    stop=(h_t == HT - 1),
                )
            nc.vector.tensor_copy(vP[:, s_t, :D], ps[:, :D])
            if s_t == KT // 2 - 1:
                yield
        nc.gpsimd.memset(vP[:, :, D:DP], 1.0)
        yield

    def gen_B(b, T):
        """agg attentions + 3 biased attentions + combine + final proj."""

        def agg_t(sd, s_t, outT):
            # transpose one normalized K_agg row-block into column layout
            pt = psT.tile([P, DT, P], bf16, name="pt_a", tag="psT")
            for d_t in range(DT):
                nc.tensor.transpose(
                    pt[:, d_t, :], sd[:, d_t * P:(d_t + 1) * P], identb
                )
            nc.vector.tensor_copy(outT[:, 0:DT, s_t * P:(s_t + 1) * P], pt)

        def agg_branch(colT_k, rowT_k, colP_k, outT):
            # scores bf16; P fp8; PV fp8 DoubleRow against the fp8 table.
            # Transposes lag the PV units by 2 so their multi-engine input
            # chain (PV -> recip -> mul) never head-of-line-blocks the PE.
            for q_c in range(QC):
                colT, rowT, colP = T[colT_k], T[rowT_k], T[colP_k]
                pch = ppool.tile([P, KT, NQ], fp8, name="pch_a", tag="pA",
                                 bufs=2)
                for m_t in range(KT):
                    ps = psA.tile([P, NQ], f32, name="ps_as", tag="psA")
                    nc.tensor.matmul(
                        ps,
                        lhsT=colT[:, 0:DT, m_t * P:(m_t + 1) * P],
                        rhs=rowT[:, 0:DT, q_c * NQ:(q_c + 1) * NQ],
                        start=True,
                        stop=True,
                        perf_mode=DR,
                    )
                    nc.scalar.activation(pch[:, m_t, :], ps, Exp, scale=SSC,
                                         bias=ebias)
                yield
                sds = []
                for q_t in range(NQ // P):
                    po = psO.tile([P, DP], f32, name="po_a", tag="psO")
                    for jm in range(KT // 2):
                        nc.tensor.matmul(
                            po,
                            lhsT=pch[:, 2 * jm:2 * jm + 2, q_t * P:(q_t + 1) * P],
                            rhs=colP[:, 2 * jm:2 * jm + 2, :],
                            start=(jm == 0),
                            stop=(jm == KT // 2 - 1),
                            perf_mode=DR,
                        )
                    rec = tmpp.tile([P, 1], f32, name="rec_a", tag="rec_a")
                    nc.vector.reciprocal(rec, po[:, D:D + 1])
                    sd = tmpp.tile([P, D], bf16, name="sd_a", tag="sd_a")
                    nc.vector.tensor_scalar_mul(sd, po[:, :D], rec)
                    sds.append(sd)
                    if q_t >= 2:
                        agg_t(sds[q_t - 2], q_c * (NQ // P) + q_t - 2, outT)
                yield
                agg_t(sds[2], q_c * (NQ // P) + 2, outT)
                agg_t(sds[3], q_c * (NQ // P) + 3, outT)
                yield

        # K_agg/K2_agg stored directly as fp8: the jump-branch logits are
        # tiny (agg rows are diffuse softmax averages), so fp8 noise there
        # perturbs the output by ~0.2% -- and it enables DoubleRow scores.
        kaT = actp.tile([P, DT, S], fp8, name="kaT", tag="kaT")
        T.update(kaT=kaT)
        yield from agg_branch("k2T8", "kT8", "k2P", kaT)   # K att K2 -> K_agg
        k2aT = actp.tile([P, DT, S], fp8, name="k2aT", tag="k2aT")
        T.update(k2aT=k2aT)
        yield from agg_branch("kT8", "k2T8", "kP", k2aT)   # K2 att K -> K2_agg

        combT = actp.tile([P, DT, S], bf16, name="combT", tag="combT")
        T.update(combT=combT)

        def comb_t(comb, s_t):
            pt = psT.tile([P, DT, P], bf16, name="pt_c", tag="psT")
            for d_t in range(DT):
                nc.tensor.transpose(
                    pt[:, d_t, :], comb[:, d_t * P:(d_t + 1) * P], identb
                )
            nc.vector.tensor_copy(combT[:, 0:DT, s_t * P:(s_t + 1) * P], pt)

        def fproj(s_t):
            # final projection of one s_t row-block, fused; output DMA goes
            # on the Act queue so the Sync queue carries only x loads (keeps
            # next-iteration x prefetch from queuing behind output drains).
            ostage = tmpp.tile([P, H], bf16, name="ostage", tag="ostage",
                               bufs=2)
            for h_c in range(HC):
                ps = psA.tile([P, NQ], f32, name="ps_o", tag="psA")
                for d_t in range(DT):
                    nc.tensor.matmul(
                        ps,
                        lhsT=combT[:, d_t, s_t * P:(s_t + 1) * P],
                        rhs=wo_sb[:, d_t, h_c * NQ:(h_c + 1) * NQ],
                        start=(d_t == 0),
                        stop=(d_t == DT - 1),
                    )
                if h_c % 2 == 0:
                    nc.scalar.copy(ostage[:, h_c * NQ:(h_c + 1) * NQ], ps)
                else:
                    nc.vector.tensor_copy(
                        ostage[:, h_c * NQ:(h_c + 1) * NQ], ps
                    )
            nc.scalar.dma_start(
                out=out_h[b, s_t * P:(s_t + 1) * P, :], in_=ostage
            )

        for q_c in range(QC):
            qT, q8, vP = T["qT"], T["q8"], T["vP"]
            kTs = (T["kT"], kaT, k2aT)
            pchs = []
            for i in range(3):
                pch = ppool.tile([P, KT, NQ], bf16, name="pch_b", tag="pB",
                                 bufs=3)
                for m_t in range(KT):
                    ps = psA.tile([P, NQ], f32, name="ps_bs", tag="psA")
                    if i == 0:
                        for d_t in range(DT):
                            nc.tensor.matmul(
                                ps,
                                lhsT=kTs[i][:, d_t, m_t * P:(m_t + 1) * P],
                                rhs=qT[:, d_t, q_c * NQ:(q_c + 1) * NQ],
                                start=(d_t == 0),
                                stop=(d_t == DT - 1),
                            )
                    else:
                        # jump branches: fp8 DoubleRow scores (K_agg x q8)
                        nc.tensor.matmul(
                            ps,
                            lhsT=kTs[i][:, 0:DT, m_t * P:(m_t + 1) * P],
                            rhs=q8[:, 0:DT, q_c * NQ:(q_c + 1) * NQ],
                            start=True,
                            stop=True,
                            perf_mode=DR,
                        )
                    et = tmpp.tile([P, NQ], bf16, name="et", tag="et", bufs=2)
                    nc.scalar.activation(et, ps, Exp, scale=SSC, bias=ebias)
                    nc.vector.tensor_mul(
                        pch[:, m_t, :], et,
                        ebT_sb[:, m_t, q_c * NQ:(q_c + 1) * NQ],
                    )
                pchs.append(pch)
                yield
            combs = []
            for q_t in range(NQ // P):
                pos = []
                for i in range(3):
                    po = psO.tile([P, DP], f32, name="po_b", tag="psO")
                    for m_t in range(KT):
                        nc.tensor.matmul(
                            po,
                            lhsT=pchs[i][:, m_t, q_t * P:(q_t + 1) * P],
                            rhs=vP[:, m_t, :],
                            start=(m_t == 0),
                            stop=(m_t == KT - 1),
                        )
                    pos.append(po)
                rec = tmpp.tile([P, 4], f32, name="rec_b", tag="rec_b")
                for i in range(3):
                    nc.vector.reciprocal(rec[:, i:i + 1], pos[i][:, D:D + 1])
                recw = tmpp.tile([P, 4], f32, name="recw", tag="recw")
                nc.vector.tensor_mul(recw, rec, wconst)
                accs = []
                for i in range(3):
                    acc = tmpp.tile([P, D], f32, name="acc", tag=f"acc{i}",
                                    bufs=2)
                    nc.scalar.mul(acc, pos[i][:, :D], recw[:, i:i + 1])
                    accs.append(acc)
                t01 = tmpp.tile([P, D], f32, name="t01", tag="t01", bufs=2)
                nc.vector.tensor_add(t01, accs[0], accs[1])
                comb = tmpp.tile([P, D], bf16, name="comb", tag="comb")
                nc.gpsimd.tensor_add(comb, t01, accs[2])
                combs.append(comb)
            yield
            # ladder: each transpose waits on its combine chain, hidden
            # under the previous block's fproj matmuls.
            s0 = q_c * (NQ // P)
            comb_t(combs[0], s0 + 0)
            fproj(s0 + 0)
            comb_t(combs[1], s0 + 1)
            fproj(s0 + 1)
            yield
            comb_t(combs[2], s0 + 2)
            fproj(s0 + 2)
            comb_t(combs[3], s0 + 3)
            fproj(s0 + 3)
            yield

    def noops(n):
        for _ in range(n):
            yield

    def interleave(*gens):
        live = list(gens)
        while live:
            for g in list(live):
                try:
                    next(g)
                except StopIteration:
                    live.remove(g)

    def xload_unit(b, T):
        xgs, xgs8 = _x_loads(nc, xpool, mybir, xT_h, xT8_h, b)
        T["xgs"] = xgs
        T["xgs8"] = xgs8
        yield

    # two emission lanes: lane1 = projection-type work (PE/DVE, no Act),
    # lane2 = attention work (Act-heavy).  Alternating units keep every
    # engine's queue supplied; noops delay lane2 until its batch-0 inputs
    # (kT/k2T/kP/k2P) are emitted.
    T0, T1 = {}, {}
    xgs0, xgs8_0 = _x_loads(nc, xpool, mybir, xT_h, xT8_h, 0)

    def lane1():
        yield from gen_A(0, xgs0, xgs8_0, T0)
        yield from xload_unit(1, T1)
        yield from gen_A(1, T1["xgs"], T1["xgs8"], T1)

    def lane2():
        yield from noops(3)
        yield from gen_B(0, T0)
        yield from gen_B(1, T1)

    interleave(lane1(), lane2())


def build_program(w_tri, reps=1):
    """reps>1 wraps the kernel body in an on-device For_i loop executing the
    identical per-iteration work back-to-back; used by test.py to measure
    per-iteration HW time with host dispatch overhead amortized away."""
    import concourse.bacc as bacc
    import concourse.tile as tile
    from concourse import mybir
    from concourse.masks import make_identity

    nc = bacc.Bacc()
    dt = mybir.dt
    bf16 = dt.bfloat16
    fp8 = dt.float8e4
    xT_h = nc.dram_tensor("xT", [BPC, H, S], bf16, kind="ExternalInput")
    xT8_h = nc.dram_tensor("xT8", [BPC, H, S], fp8, kind="ExternalInput")
    wq_h = nc.dram_tensor("wq", [H, D], bf16, kind="ExternalInput")
    wk_h = nc.dram_tensor("wk", [H, D], bf16, kind="ExternalInput")
    wk2_h = nc.dram_tensor("wk2", [H, D], fp8, kind="ExternalInput")
    wv_h = nc.dram_tensor("wv", [H, D], bf16, kind="ExternalInput")
    wo_h = nc.dram_tensor("wo", [D, H], bf16, kind="ExternalInput")
    ebT_h = nc.dram_tensor("ebT", [S, S], bf16, kind="ExternalInput")
    out_h = nc.dram_tensor("out", [BPC, S, H], bf16, kind="ExternalOutput")
    handles = (xT_h, xT8_h, wq_h, wk_h, wk2_h, wv_h, wo_h, ebT_h, out_h)

    with ExitStack() as ctx:
        tc = ctx.enter_context(tile.TileContext(nc))
        pools, cst = _consts(nc, tc, ctx, mybir, make_identity, handles, w_tri)
        if reps == 1:
            _body(nc, tc, mybir, handles, pools, cst)
        else:
            # For_i inserts a full cross-engine barrier (+ several-us branch
            # re-launch) at every loop-back; unrolling UF bodies per hardware
            # iteration amortizes that boundary cost and lets the x-load DMA
            # of body k+1 prefetch during body k's tail.
            uf = 8 if reps % 8 == 0 else (4 if reps % 4 == 0 else
                                          (2 if reps % 2 == 0 else 1))
            with tc.For_i(0, reps // uf):
                for _ in range(uf):
                    _body(nc, tc, mybir, handles, pools, cst)
    nc.compile()
    return nc


def prep_inputs(hidden_states, Wq, Wk, Wk2, Wv, Wo, triadic_weight, pos_bias):
    f32 = np.float32
    bf16 = ml_dtypes.bfloat16
    fp8 = ml_dtypes.float8_e4m3  # TRN float8e4 (|x| <= 240: same encoding)

    t = np.asarray(triadic_weight, dtype=np.float64)
    e = np.exp(t - t.max())
    w_tri = (e / e.sum()).astype(f32)

    wq_np = np.asarray(Wq, f32).astype(bf16)
    wk_np = np.asarray(Wk, f32).astype(bf16)
    wk2_np = np.asarray(Wk2, f32).astype(fp8)
    wv_np = np.asarray(Wv, f32).astype(bf16)
    wo_np = np.asarray(Wo, f32).astype(bf16)
    ebT_np = np.exp(np.asarray(pos_bias, f32).T).astype(bf16)
    hs = np.asarray(hidden_states, f32)
    xTs = [
        np.ascontiguousarray(hs[c * BPC:(c + 1) * BPC].transpose(0, 2, 1)).astype(bf16)
        for c in range(NCORES)
    ]
    xT8s = [x.astype(fp8) for x in xTs]
    in_maps = [
        {
            "xT": xTs[c],
            "xT8": xT8s[c],
            "wq": wq_np,
            "wk": wk_np,
            "wk2": wk2_np,
            "wv": wv_np,
            "wo": wo_np,
            "ebT": ebT_np,
        }
        for c in range(NCORES)
    ]
    return w_tri, in_maps


def kernel(hidden_states, Wq, Wk, Wk2, Wv, Wo, triadic_weight, pos_bias):
    global LAST_RESULTS
    from concourse.bass_utils import run_bass_kernel_spmd

    f32 = np.float32
    w_tri, in_maps = prep_inputs(
        hidden_states, Wq, Wk, Wk2, Wv, Wo, triadic_weight, pos_bias
    )
    nc = build_program(w_tri)

    if os.environ.get("KERNEL_BUILD_ONLY"):
        return np.zeros((B, S, H), f32)

    res = run_bass_kernel_spmd(nc, in_maps, core_ids=list(range(NCORES)))
    LAST_RESULTS = res
    if res.exec_time_ns:
        print(f"HW exec time: {res.exec_time_ns} ns")
    out = np.concatenate([r["out"] for r in res.results], axis=0)
    return np.ascontiguousarray(out.astype(f32))



# revision 30
# speedup vs baseline: 1.8152x; 1.8152x over previous
"""JumpAttention Trainium2 kernel (bf16 main path, fp8 agg garnish).

Problem: B=16, S=1024, H=2048, D=256.
  Q/K/K2/V = hs @ W*, 3 biased attentions + 2 aggregation attentions,
  triadic-weighted combine, output projection by Wo.

Strategy:
  - Data-parallel over batch: 2 batches per core on 8 cores; weights and
    pos_bias replicated.
  - bf16 matmuls with fp32 PSUM accumulation on the main path (fp8 there
    fails the accuracy gate: diffuse attention averages the signal down
    as fast as the noise, so per-element fp8 noise stays 2-4% of the
    output).  The aggregation branches are nearly flat softmaxes and
    tolerate fp8: their P tiles and K/K2 value tables run fp8-e4m3 with
    DoubleRow perf mode (half the PE instructions and cycles there).
  - The jump branches are noise-tolerant (K_agg rows are diffuse softmax
    averages, so jump logits are ~0.05-magnitude): their score matmuls run
    fp8 DoubleRow (K_agg/K2_agg stored fp8, fp8 copy of Q), and the K2
    projection runs fully fp8 DR (x8 and Wk2 shipped fp8 from host) since
    K2 only feeds fp8 consumers.  Measured cost: rel_err 5.5e-3 -> 6.2e-3.
  - Scores are computed transposed (s^T[k, q]) so that exp(s^T) directly
    serves as the stationary operand of the P@V matmul - no P transpose.
  - Softmax has no max-subtraction; exp(s - 3) keeps the fp8 agg P in
    range (the shift cancels in softmax); the denominator comes from a
    ones-column appended to the value table (row-sums appear as one
    extra output column of the P@V matmul).
  - pos_bias folded multiplicatively: exp(s + b) = exp(s) * exp(b), with
    exp(pos_bias^T) precomputed on host in bf16 (the multiply runs on
    DVE in its 2x all-SBUF bf16 mode).
  - softmax(triadic_weight) computed on host, baked in as immediates.
  - Two-batch software pipeline: emission is interleaved in two lanes so
    batch 1's projection work (PE/DVE, no Act) fills the engine gaps
    under batch 0's attention (Act-heavy), and each batch's final
    projection is fused into its attention units (no drain tail).
  - Output DMAs issue from the Act queue so the Sync queue carries only
    x loads (keeps x prefetch from queuing behind output drains).
  - Constants (weights, exp-bias table, identity) load outside the
    timing loop; build_program(reps=N) wraps the per-iteration body in
    an on-device For_i loop so test.py can measure per-iteration HW time
    with host dispatch overhead amortized away.  The For_i loop-back is a
    full cross-engine barrier (~18us with the boundary x-load stall), so
    the body is unrolled 8x inside the loop to amortize it.
"""

import os
from contextlib import ExitStack

import numpy as np
import ml_dtypes

B, S, H, D = 16, 1024, 2048, 256
NCORES = 8
BPC = B // NCORES  # batches per core
P = 128
HT = H // P   # 16 h-tiles
KT = S // P   # 8 s-tiles
DT = D // P   # 2 d-tiles
NQ = 512      # moving free dim (q) chunk
QC = S // NQ  # 2 q chunks
HC = H // NQ  # 4 h chunks in final projection
XG = 8        # h-tiles per x load group
DP = D + 2    # value-table width: D cols + ones col + pad (8B-aligned PSUM rows)

EXPB = -3.0   # exp argument shift (cancels in softmax; keeps fp8 P in range)

LAST_RESULTS = None


def _consts(nc, tc, ctx, mybir, make_identity, handles, w_tri):
    """Pools + loop-invariant constants (emitted outside the reps loop)."""
    xT_h, xT8_h, wq_h, wk_h, wk2_h, wv_h, wo_h, ebT_h, out_h = handles
    dt = mybir.dt
    bf16 = dt.bfloat16
    f32 = dt.float32
    fp8 = dt.float8e4

    pools = dict(
        consts=ctx.enter_context(tc.tile_pool(name="consts", bufs=1)),
        xpool=ctx.enter_context(tc.tile_pool(name="xpool", bufs=2)),
        actp=ctx.enter_context(tc.tile_pool(name="actp", bufs=1)),
        ppool=ctx.enter_context(tc.tile_pool(name="ppool", bufs=1)),
        tmpp=ctx.enter_context(tc.tile_pool(name="tmpp", bufs=4)),
        psA=ctx.enter_context(tc.tile_pool(name="psA", bufs=4, space="PSUM")),
        psO=ctx.enter_context(tc.tile_pool(name="psO", bufs=3, space="PSUM")),
        psT=ctx.enter_context(tc.tile_pool(name="psT", bufs=1, space="PSUM")),
    )
    consts = pools["consts"]

    wq_sb = consts.tile([P, HT, D], bf16, name="wq_sb")
    wk_sb = consts.tile([P, HT, D], bf16, name="wk_sb")
    wk2_sb = consts.tile([P, HT, D], fp8, name="wk2_sb")
    wv_sb = consts.tile([P, HT, D], bf16, name="wv_sb")
    for t_, h_ in ((wq_sb, wq_h), (wk_sb, wk_h), (wk2_sb, wk2_h), (wv_sb, wv_h)):
        nc.sync.dma_start(out=t_, in_=h_[:].rearrange("(t p) d -> p t d", p=P))
    wo_sb = consts.tile([P, DT, H], bf16, name="wo_sb")
    nc.sync.dma_start(out=wo_sb, in_=wo_h[:].rearrange("(t p) h -> p t h", p=P))
    ebT_sb = consts.tile([P, KT, S], bf16, name="ebT_sb")
    nc.sync.dma_start(out=ebT_sb, in_=ebT_h[:].rearrange("(t p) q -> p t q", p=P))
    identb = consts.tile([P, P], bf16, name="identb")
    make_identity(nc, identb)
    ident8 = consts.tile([P, P], fp8, name="ident8")
    nc.vector.tensor_copy(ident8, identb)
    ebias = consts.tile([P, 1], f32, name="ebias")
    nc.gpsimd.memset(ebias, EXPB)
    wconst = consts.tile([P, 4], f32, name="wconst")
    for _i, _w in enumerate((float(w_tri[0]), float(w_tri[1]),
                             float(w_tri[2]), 0.0)):
        nc.gpsimd.memset(wconst[:, _i:_i + 1], _w)
    cst = dict(wq_sb=wq_sb, wk_sb=wk_sb, wk2_sb=wk2_sb, wv_sb=wv_sb,
               wo_sb=wo_sb, ebT_sb=ebT_sb, identb=identb, ident8=ident8,
               ebias=ebias, wconst=wconst)
    return pools, cst



def _x_loads(nc, xpool, mybir, xT_h, xT8_h, b):
    xgs, xgs8 = [], []
    for g in range(HT // XG):
        xg = xpool.tile([P, XG, S], mybir.dt.bfloat16, name="xg", tag="xg")
        nc.sync.dma_start(
            out=xg,
            in_=xT_h[b, g * XG * P:(g + 1) * XG * P, :].rearrange(
                "(t p) q -> p t q", p=P
            ),
        )
        xgs.append(xg)
        xg8 = xpool.tile([P, XG, S], mybir.dt.float8e4, name="xg8", tag="xg8")
        nc.sync.dma_start(
            out=xg8,
            in_=xT8_h[b, g * XG * P:(g + 1) * XG * P, :].rearrange(
                "(t p) q -> p t q", p=P
            ),
        )
        xgs8.append(xg8)
    return xgs, xgs8


def _body(nc, tc, mybir, handles, pools, cst):
    """One full iteration of the kernel (both batches)."""
    xT_h, xT8_h, wq_h, wk_h, wk2_h, wv_h, wo_h, ebT_h, out_h = handles
    dt = mybir.dt
    bf16 = dt.bfloat16
    f32 = dt.float32
    fp8 = dt.float8e4
    DR = mybir.MatmulPerfMode.DoubleRow
    Exp = mybir.ActivationFunctionType.Exp
    SSC = float(D) ** -0.5

    xpool, actp, ppool, tmpp = (pools[k] for k in
                                ("xpool", "actp", "ppool", "tmpp"))
    psA, psO, psT = (pools[k] for k in ("psA", "psO", "psT"))
    (wq_sb, wk_sb, wk2_sb, wv_sb, wo_sb, ebT_sb, identb, ident8, ebias,
     wconst) = (
        cst[k] for k in ("wq_sb", "wk_sb", "wk2_sb", "wv_sb", "wo_sb",
                         "ebT_sb", "identb", "ident8", "ebias", "wconst"))

    def gen_A(b, xgs, xgs8, T):
        """x projections, V, K'/K2' fp8 row tables. No Act-engine work."""
        xts = [xgs[h_t // XG][:, h_t % XG, :] for h_t in range(HT)]

        kT = actp.tile([P, DT, S], bf16, name="kT", tag="kT", bufs=2)
        qT = actp.tile([P, DT, S], bf16, name="qT", tag="qT", bufs=2)
        T.update(qT=qT, kT=kT)

        def proj_cols(dst, w_sb):
            # finer emission granularity: yield after the first half so the
            # attention lane's units interleave at ~7us instead of ~15us.
            for d_t in range(DT):
                for q_c in range(QC):
                    ps = psA.tile([P, NQ], f32, name="ps_proj", tag="psA")
                    for h_t in range(HT):
                        nc.tensor.matmul(
                            ps,
                            lhsT=w_sb[:, h_t, d_t * P:(d_t + 1) * P],
                            rhs=xts[h_t][:, q_c * NQ:(q_c + 1) * NQ],
                            start=(h_t == 0),
                            stop=(h_t == HT - 1),
                        )
                    nc.vector.tensor_copy(dst[:, d_t, q_c * NQ:(q_c + 1) * NQ], ps)
                if d_t == 0:
                    yield

        kT8 = actp.tile([P, DT, S], fp8, name="kT8", tag="kT8", bufs=2)
        k2T8 = actp.tile([P, DT, S], fp8, name="k2T8", tag="k2T8", bufs=2)
        T.update(kT8=kT8, k2T8=k2T8)
        yield from proj_cols(kT, wk_sb)
        nc.vector.tensor_copy(kT8, kT)
        yield
        # K2 projection fully in fp8 DoubleRow (x8 @ Wk2_8): K2 only feeds
        # fp8 consumers (agg scores + fp8 value table), which tolerate the
        # extra quantization noise; halves this projection's PE cycles.
        for d_t in range(DT):
            for q_c in range(QC):
                ps = psA.tile([P, NQ], f32, name="ps_proj8", tag="psA")
                for hp in range(HT // 2):
                    g, j = hp // (XG // 2), hp % (XG // 2)
                    nc.tensor.matmul(
                        ps,
                        lhsT=wk2_sb[:, 2 * hp:2 * hp + 2, d_t * P:(d_t + 1) * P],
                        rhs=xgs8[g][:, 2 * j:2 * j + 2, q_c * NQ:(q_c + 1) * NQ],
                        start=(hp == 0),
                        stop=(hp == HT // 2 - 1),
                        perf_mode=DR,
                    )
                nc.vector.tensor_copy(
                    k2T8[:, d_t, q_c * NQ:(q_c + 1) * NQ], ps
                )
        yield

        # K', K2' row-layout fp8 value tables (+ ones col) for the agg PV;
        # transposes of K^T/K2^T, fp8 conversion in the copy.
        kP = actp.tile([P, KT, DP], fp8, name="kP", tag="kP", bufs=2)
        k2P = actp.tile([P, KT, DP], fp8, name="k2P", tag="k2P", bufs=2)
        T.update(kP=kP, k2P=k2P)
        for s_t in range(KT):
            pt = psT.tile([P, DT, P], bf16, name="pt_k", tag="psT")
            for d_t in range(DT):
                nc.tensor.transpose(
                    pt[:, d_t, :], kT[:, d_t, s_t * P:(s_t + 1) * P], identb
                )
            nc.vector.tensor_copy(kP[:, s_t, :D], pt)
        nc.gpsimd.memset(kP[:, :, D:DP], 1.0)
        for s_t in range(KT):
            # fp8 transpose-mode requires output element step 2
            pt = psT.tile([P, DT, 2 * P], fp8, name="pt_k8", tag="psT")
            for d_t in range(DT):
                nc.tensor.transpose(
                    pt[:, d_t, 0:2 * P:2],
                    k2T8[:, d_t, s_t * P:(s_t + 1) * P], ident8
                )
            nc.vector.tensor_copy(k2P[:, s_t, :D], pt[:, :, 0:2 * P:2])
        nc.gpsimd.memset(k2P[:, :, D:DP], 1.0)
        yield

        yield from proj_cols(qT, wq_sb)
        # fp8 Q for the jump-branch scores (their logits are tiny -- K_agg
        # rows are diffuse softmax averages -- so fp8 noise is negligible
        # there; the std branch keeps bf16 Q).
        q8 = actp.tile([P, DT, S], fp8, name="q8", tag="q8", bufs=2)
        nc.vector.tensor_copy(q8, qT)
        T.update(q8=q8)
        yield

        # V' : [s-part, d + ones-column] bf16 (x as stationary)
        vP = actp.tile([P, KT, DP], bf16, name="vP", tag="vP", bufs=2)
        T.update(vP=vP)
        for s_t in range(KT):
            ps = psA.tile([P, NQ], f32, name="ps_v", tag="psA")
            for h_t in range(HT):
                nc.tensor.matmul(
                    ps[:, :D],
                    lhsT=xts[h_t][:, s_t * P:(s_t + 1) * P],
                    rhs=wv_sb[:, h_t, :],
                    start=(h_t == 0),
                    stop=(h_t == HT - 1),
                )
            nc.vector.tensor_copy(vP[:, s_t, :D], ps[:, :D])
            if s_t == KT // 2 - 1:
                yield
        nc.gpsimd.memset(vP[:, :, D:DP], 1.0)
        yield

    def gen_B(b, T):
        """agg attentions + 3 biased attentions + combine + final proj."""

        def agg_t(sd, s_t, outT):
            # transpose one normalized K_agg row-block into column layout
            pt = psT.tile([P, DT, P], bf16, name="pt_a", tag="psT")
            for d_t in range(DT):
                nc.tensor.transpose(
                    pt[:, d_t, :], sd[:, d_t * P:(d_t + 1) * P], identb
                )
            nc.vector.tensor_copy(outT[:, 0:DT, s_t * P:(s_t + 1) * P], pt)

        def agg_branch(colT_k, rowT_k, colP_k, outT):
            # scores bf16; P fp8; PV fp8 DoubleRow against the fp8 table.
            # Transposes lag the PV units by 2 so their multi-engine input
            # chain (PV -> recip -> mul) never head-of-line-blocks the PE.
            for q_c in range(QC):
                colT, rowT, colP = T[colT_k], T[rowT_k], T[colP_k]
                pch = ppool.tile([P, KT, NQ], fp8, name="pch_a", tag="pA",
                                 bufs=2)
                for m_t in range(KT):
                    ps = psA.tile([P, NQ], f32, name="ps_as", tag="psA")
                    nc.tensor.matmul(
                        ps,
                        lhsT=colT[:, 0:DT, m_t * P:(m_t + 1) * P],
                        rhs=rowT[:, 0:DT, q_c * NQ:(q_c + 1) * NQ],
                        start=True,
                        stop=True,
                        perf_mode=DR,
                    )
                    nc.scalar.activation(pch[:, m_t, :], ps, Exp, scale=SSC,
                                         bias=ebias)
                yield
                sds = []
                for q_t in range(NQ // P):
                    po = psO.tile([P, DP], f32, name="po_a", tag="psO")
                    for jm in range(KT // 2):
                        nc.tensor.matmul(
                            po,
                            lhsT=pch[:, 2 * jm:2 * jm + 2, q_t * P:(q_t + 1) * P],
                            rhs=colP[:, 2 * jm:2 * jm + 2, :],
                            start=(jm == 0),
                            stop=(jm == KT // 2 - 1),
                            perf_mode=DR,
                        )
                    rec = tmpp.tile([P, 1], f32, name="rec_a", tag="rec_a")
                    nc.vector.reciprocal(rec, po[:, D:D + 1])
                    sd = tmpp.tile([P, D], bf16, name="sd_a", tag="sd_a")
                    nc.vector.tensor_scalar_mul(sd, po[:, :D], rec)
                    sds.append(sd)
                    if q_t >= 2:
                        agg_t(sds[q_t - 2], q_c * (NQ // P) + q_t - 2, outT)
                yield
                agg_t(sds[2], q_c * (NQ // P) + 2, outT)
                agg_t(sds[3], q_c * (NQ // P) + 3, outT)
                yield

        # K_agg/K2_agg stored directly as fp8: the jump-branch logits are
        # tiny (agg rows are diffuse softmax averages), so fp8 noise there
        # perturbs the output by ~0.2% -- and it enables DoubleRow scores.
        kaT = actp.tile([P, DT, S], fp8, name="kaT", tag="kaT")
        T.update(kaT=kaT)
        yield from agg_branch("k2T8", "kT8", "k2P", kaT)   # K att K2 -> K_agg
        k2aT = actp.tile([P, DT, S], fp8, name="k2aT", tag="k2aT")
        T.update(k2aT=k2aT)
        yield from agg_branch("kT8", "k2T8", "kP", k2aT)   # K2 att K -> K2_agg

        combT = actp.tile([P, DT, S], bf16, name="combT", tag="combT")
        T.update(combT=combT)

        def comb_t(comb, s_t):
            pt = psT.tile([P, DT, P], bf16, name="pt_c", tag="psT")
            for d_t in range(DT):
                nc.tensor.transpose(
                    pt[:, d_t, :], comb[:, d_t * P:(d_t + 1) * P], identb
                )
            nc.vector.tensor_copy(combT[:, 0:DT, s_t * P:(s_t + 1) * P], pt)

        def fproj(s_t):
            # final projection of one s_t row-block, fused; output DMA goes
            # on the Act queue so the Sync queue carries only x loads (keeps
            # next-iteration x prefetch from queuing behind output drains).
            ostage = tmpp.tile([P, H], bf16, name="ostage", tag="ostage",
                               bufs=2)
            for h_c in range(HC):
                ps = psA.tile([P, NQ], f32, name="ps_o", tag="psA")
                for d_t in range(DT):
                    nc.tensor.matmul(
                        ps,
                        lhsT=combT[:, d_t, s_t * P:(s_t + 1) * P],
                        rhs=wo_sb[:, d_t, h_c * NQ:(h_c + 1) * NQ],
                        start=(d_t == 0),
                        stop=(d_t == DT - 1),
                    )
                if h_c % 2 == 0:
                    nc.scalar.copy(ostage[:, h_c * NQ:(h_c + 1) * NQ], ps)
                else:
                    nc.vector.tensor_copy(
                        ostage[:, h_c * NQ:(h_c + 1) * NQ], ps
                    )
            nc.scalar.dma_start(
                out=out_h[b, s_t * P:(s_t + 1) * P, :], in_=ostage
            )

        for q_c in range(QC):
            qT, q8, vP = T["qT"], T["q8"], T["vP"]
            kTs = (T["kT"], kaT, k2aT)
            pchs = []
            for i in range(3):
                pch = ppool.tile([P, KT, NQ], bf16, name="pch_b", tag="pB",
                                 bufs=3)
                for m_t in range(KT):
                    ps = psA.tile([P, NQ], f32, name="ps_bs", tag="psA")
                    if i == 0:
                        for d_t in range(DT):
                            nc.tensor.matmul(
                                ps,
                                lhsT=kTs[i][:, d_t, m_t * P:(m_t + 1) * P],
                                rhs=qT[:, d_t, q_c * NQ:(q_c + 1) * NQ],
                                start=(d_t == 0),
                                stop=(d_t == DT - 1),
                            )
                    else:
                        # jump branches: fp8 DoubleRow scores (K_agg x q8)
                        nc.tensor.matmul(
                            ps,
                            lhsT=kTs[i][:, 0:DT, m_t * P:(m_t + 1) * P],
                            rhs=q8[:, 0:DT, q_c * NQ:(q_c + 1) * NQ],
                            start=True,
                            stop=True,
                            perf_mode=DR,
                        )
                    et = tmpp.tile([P, NQ], bf16, name="et", tag="et", bufs=2)
                    nc.scalar.activation(et, ps, Exp, scale=SSC, bias=ebias)
                    nc.vector.tensor_mul(
                        pch[:, m_t, :], et,
                        ebT_sb[:, m_t, q_c * NQ:(q_c + 1) * NQ],
                    )
                pchs.append(pch)
                yield
            combs = []
            for q_t in range(NQ // P):
                pos = []
                for i in range(3):
                    po = psO.tile([P, DP], f32, name="po_b", tag="psO")
                    for m_t in range(KT):
                        nc.tensor.matmul(
                            po,
                            lhsT=pchs[i][:, m_t, q_t * P:(q_t + 1) * P],
                            rhs=vP[:, m_t, :],
                            start=(m_t == 0),
                            stop=(m_t == KT - 1),
                        )
                    pos.append(po)
                rec = tmpp.tile([P, 4], f32, name="rec_b", tag="rec_b")
                for i in range(3):
                    nc.vector.reciprocal(rec[:, i:i + 1], pos[i][:, D:D + 1])
                recw = tmpp.tile([P, 4], f32, name="recw", tag="recw")
                nc.vector.tensor_mul(recw, rec, wconst)
                accs = []
                for i in range(3):
                    acc = tmpp.tile([P, D], f32, name="acc", tag=f"acc{i}",
                                    bufs=2)
                    nc.scalar.mul(acc, pos[i][:, :D], recw[:, i:i + 1])
                    accs.append(acc)
                t01 = tmpp.tile([P, D], f32, name="t01", tag="t01", bufs=2)
                nc.gpsimd.tensor_add(t01, accs[0], accs[1])
                comb = tmpp.tile([P, D], bf16, name="comb", tag="comb")
                nc.gpsimd.tensor_add(comb, t01, accs[2])
                combs.append(comb)
                # transposes lag the PV/combine units by 2 so the chain
                # (PV -> recip -> mul -> add -> add) is hidden under the
                # next units' PE work.
                if q_t >= 2:
                    comb_t(combs[q_t - 2], q_c * (NQ // P) + q_t - 2)
            yield
            s0 = q_c * (NQ // P)
            comb_t(combs[2], s0 + 2)
            fproj(s0 + 0)
            comb_t(combs[3], s0 + 3)
            fproj(s0 + 1)
            yield
            fproj(s0 + 2)
            fproj(s0 + 3)
            yield

    def noops(n):
        for _ in range(n):
            yield

    def interleave(*gens):
        live = list(gens)
        while live:
            for g in list(live):
                try:
                    next(g)
                except StopIteration:
                    live.remove(g)

    def xload_unit(b, T):
        xgs, xgs8 = _x_loads(nc, xpool, mybir, xT_h, xT8_h, b)
        T["xgs"] = xgs
        T["xgs8"] = xgs8
        yield

    # two emission lanes: lane1 = projection-type work (PE/DVE, no Act),
    # lane2 = attention work (Act-heavy).  Alternating units keep every
    # engine's queue supplied; noops delay lane2 until its batch-0 inputs
    # (kT/k2T/kP/k2P) are emitted.
    T0, T1 = {}, {}
    xgs0, xgs8_0 = _x_loads(nc, xpool, mybir, xT_h, xT8_h, 0)

    def lane1():
        yield from gen_A(0, xgs0, xgs8_0, T0)
        yield from xload_unit(1, T1)
        yield from gen_A(1, T1["xgs"], T1["xgs8"], T1)

    def lane2():
        yield from noops(3)
        yield from gen_B(0, T0)
        yield from gen_B(1, T1)

    interleave(lane1(), lane2())


def build_program(w_tri, reps=1):
    """reps>1 wraps the kernel body in an on-device For_i loop executing the
    identical per-iteration work back-to-back; used by test.py to measure
    per-iteration HW time with host dispatch overhead amortized away."""
    import concourse.bacc as bacc
    import concourse.tile as tile
    from concourse import mybir
    from concourse.masks import make_identity

    nc = bacc.Bacc()
    dt = mybir.dt
    bf16 = dt.bfloat16
    fp8 = dt.float8e4
    xT_h = nc.dram_tensor("xT", [BPC, H, S], bf16, kind="ExternalInput")
    xT8_h = nc.dram_tensor("xT8", [BPC, H, S], fp8, kind="ExternalInput")
    wq_h = nc.dram_tensor("wq", [H, D], bf16, kind="ExternalInput")
    wk_h = nc.dram_tensor("wk", [H, D], bf16, kind="ExternalInput")
    wk2_h = nc.dram_tensor("wk2", [H, D], fp8, kind="ExternalInput")
    wv_h = nc.dram_tensor("wv", [H, D], bf16, kind="ExternalInput")
    wo_h = nc.dram_tensor("wo", [D, H], bf16, kind="ExternalInput")
    ebT_h = nc.dram_tensor("ebT", [S, S], bf16, kind="ExternalInput")
    out_h = nc.dram_tensor("out", [BPC, S, H], bf16, kind="ExternalOutput")
    handles = (xT_h, xT8_h, wq_h, wk_h, wk2_h, wv_h, wo_h, ebT_h, out_h)

    with ExitStack() as ctx:
        tc = ctx.enter_context(tile.TileContext(nc))
        pools, cst = _consts(nc, tc, ctx, mybir, make_identity, handles, w_tri)
        if reps == 1:
            _body(nc, tc, mybir, handles, pools, cst)
        else:
            # For_i inserts a full cross-engine barrier (+ several-us branch
            # re-launch) at every loop-back; unrolling UF bodies per hardware
            # iteration amortizes that boundary cost and lets the x-load DMA
            # of body k+1 prefetch during body k's tail.
            uf = 8 if reps % 8 == 0 else (4 if reps % 4 == 0 else
                                          (2 if reps % 2 == 0 else 1))
            with tc.For_i(0, reps // uf):
                for _ in range(uf):
                    _body(nc, tc, mybir, handles, pools, cst)
    nc.compile()
    return nc


def prep_inputs(hidden_states, Wq, Wk, Wk2, Wv, Wo, triadic_weight, pos_bias):
    f32 = np.float32
    bf16 = ml_dtypes.bfloat16
    fp8 = ml_dtypes.float8_e4m3  # TRN float8e4 (|x| <= 240: same encoding)

    t = np.asarray(triadic_weight, dtype=np.float64)
    e = np.exp(t - t.max())
    w_tri = (e / e.sum()).astype(f32)

    wq_np = np.asarray(Wq, f32).astype(bf16)
    wk_np = np.asarray(Wk, f32).astype(bf16)
    wk2_np = np.asarray(Wk2, f32).astype(fp8)
    wv_np = np.asarray(Wv, f32).astype(bf16)
    wo_np = np.asarray(Wo, f32).astype(bf16)
    ebT_np = np.exp(np.asarray(pos_bias, f32).T).astype(bf16)
    hs = np.asarray(hidden_states, f32)
    xTs = [
        np.ascontiguousarray(hs[c * BPC:(c + 1) * BPC].transpose(0, 2, 1)).astype(bf16)
        for c in range(NCORES)
    ]
    xT8s = [x.astype(fp8) for x in xTs]
    in_maps = [
        {
            "xT": xTs[c],
            "xT8": xT8s[c],
            "wq": wq_np,
            "wk": wk_np,
            "wk2": wk2_np,
            "wv": wv_np,
            "wo": wo_np,
            "ebT": ebT_np,
        }
        for c in range(NCORES)
    ]
    return w_tri, in_maps


def kernel(hidden_states, Wq, Wk, Wk2, Wv, Wo, triadic_weight, pos_bias):
    global LAST_RESULTS
    from concourse.bass_utils import run_bass_kernel_spmd

    f32 = np.float32
    w_tri, in_maps = prep_inputs(
        hidden_states, Wq, Wk, Wk2, Wv, Wo, triadic_weight, pos_bias
    )
    nc = build_program(w_tri)

    if os.environ.get("KERNEL_BUILD_ONLY"):
        return np.zeros((B, S, H), f32)

    res = run_bass_kernel_spmd(nc, in_maps, core_ids=list(range(NCORES)))
    LAST_RESULTS = res
    if res.exec_time_ns:
        print(f"HW exec time: {res.exec_time_ns} ns")
    out = np.concatenate([r["out"] for r in res.results], axis=0)
    return np.ascontiguousarray(out.astype(f32))

